# revision 50
# baseline (speedup 1.0000x reference)
"""NeuralObliviousDecisionTree kernel for 8x Trainium2 NeuronCores.

Math (per batch shard):
    z[b, k, t]      = x @ W[k] + bias[k]                       (k = tree level)
    q[b, k, t, j]   = sigmoid((1-2j) * z)                      (j = branch side)
    path[b, t, l]   = prod_k q[b, k, t, bit_k(l)]
    out[b, d]       = (1/T) sum_{t,l} path[b, t, l] * leaf[t, l, d]

Implemented in the log domain so every heavy stage is a matmul or an
activation-engine pass (no vector-engine product expansion, no transposes
of the big path tensor):
    MM1: logits'.T = W'.T @ x.T  with W' columns (pair, tloc, k, j) holding
         (2j-1)*W[k,:,t]  -> PSUM [(rows), b]; each pair gets a 32-row block
         (24 live rows + 8 zero-padded) so every matmul stays 128-row aligned
    ACT: NLQ = Ln(1 + Exp(-(logits' + bias')))   ( = softplus(-u) = -log q;
         built from Exp and Ln because the HW activation tables have no
         softplus, and {Exp, Ln, Copy, Identity} share one table -> no
         activation-table reloads anywhere in the kernel )
    MM2: U = S0'.T @ NLQ  per tree-pair      ( = -log path, [128 rows, b] )
         with one of 4 S0 variants selecting the pair's 32-row block
    ACT: path = exp(-U)
    MM3: out.T += leafpair.T @ path          (PSUM accumulation over pairs)

Sharding: batch 16384 split evenly across 8 cores; all parameters replicated.
"""

import os
import numpy as np
from contextlib import ExitStack

import concourse.bass as bass
import concourse.tile as tile
from concourse import bacc, mybir
from concourse.bass import ts

f32 = mybir.dt.float32
f32r = mybir.dt.float32r
bf16 = mybir.dt.bfloat16
AF = mybir.ActivationFunctionType

# Matmul operand dtype. float32r streams 1 column/cycle on TRN2 (plain
# float32 pays a 4x "two half-speed matmuls" penalty). The BIR verifier
# requires every producer of an fp32r matmul operand to emit fp32r itself,
# so the affected DRAM tensors and SBUF tiles are declared float32r
# natively (same 4-byte layout; numpy side stays float32).
MM_DT = f32r

N_CORES = 8
B_FULL, F = 16384, 512
DEPTH, T, D, L = 6, 64, 64, 64
B = B_FULL // N_CORES        # 2048 rows per core
NBT = B // 128               # 16 partition tiles of the batch
NFC = F // 128               # 4 feature chunks (MM1 contraction tiles)
NBC = 4                      # batch chunks for PSUM-width-512 stages
BC = B // NBC                # 512
PBLK = 32                    # rows per tree-pair block (24 live + 8 pad)
MROWS = 128                  # NLQ tile rows = 4 tree-pair blocks
NMT = (T // 2 * PBLK) // MROWS  # 8 NLQ row tiles
NPAIR = T // 2               # 32 tree pairs
WCOLS = T // 2 * PBLK        # 1024 columns of the doubled weight matrix


def _build_kernel(nc, aps, repeat=1, hw_loop=0):
    x_d, wp_d, bp_d, s0_d, leaf_d, ident_d, out_d = aps
    tc = tile.TileContext(nc)
    with tc, ExitStack() as ctx:
        const = ctx.enter_context(tc.tile_pool(name="const", bufs=1))
        xin = ctx.enter_context(tc.tile_pool(name="xin", bufs=3))
        nlqp = ctx.enter_context(tc.tile_pool(name="nlqp", bufs=16))
        expp = ctx.enter_context(tc.tile_pool(name="expp", bufs=3))
        pathp = ctx.enter_context(tc.tile_pool(name="pathp", bufs=4))
        outp = ctx.enter_context(tc.tile_pool(name="outp", bufs=2))
        psA = ctx.enter_context(tc.tile_pool(name="psA", bufs=2, space="PSUM"))
        psU = ctx.enter_context(tc.tile_pool(name="psU", bufs=2, space="PSUM"))
        psO = ctx.enter_context(tc.tile_pool(name="psO", bufs=1, space="PSUM"))
        psT = ctx.enter_context(tc.tile_pool(name="psT", bufs=1, space="PSUM"))

        ident = const.tile([128, 128], f32)
        nc.sync.dma_start(ident[:], ident_d)
        wp_r = wp_d.rearrange("(c p) (m w) -> m p c w", p=128, m=NMT)
        wpm = [
            const.tile([128, NFC, MROWS], MM_DT, name=f"wp{m}") for m in range(NMT)
        ]
        for m in range(NMT):
            nc.sync.dma_start(wpm[m][:], wp_r[m])
        bp = const.tile([MROWS, NMT], f32)
        nc.sync.dma_start(bp[:], bp_d)
        s0 = const.tile([128, 4, 128], MM_DT)
        nc.sync.dma_start(s0[:], s0_d.rearrange("(q p) w -> p q w", p=128))
        leaf = const.tile([128, NPAIR, D], MM_DT)
        nc.sync.dma_start(leaf[:], leaf_d.rearrange("(g p) d -> p g d", p=128))

        xts = [
            const.tile([128, NFC, BC], MM_DT, name=f"xt{n}") for n in range(NBC)
        ]
        outT = const.tile([64, B], f32)

        x_r = x_d.rearrange("(n i p) f -> n i p f", p=128, i=4)

        def stage1(n, pool=None):
            # x.T for batch chunk n via PE transposes (plain f32).
            # Chunk 0 runs before any MM1 work, so it may use the then-idle
            # psA pool (2 bufs) to pipeline transpose vs copy-out.
            pool, pname = pool or (psT, "pt")
            for i4 in range(4):
                xtile = xin.tile([128, F], f32, name="xtile")
                nc.gpsimd.dma_start(xtile[:], x_r[n, i4])
                for c in range(NFC):
                    pt = pool.tile([128, 512], f32, name=pname)[:, :128]
                    nc.tensor.transpose(pt, xtile[:, ts(c, 128)], ident[:])
                    nc.vector.tensor_copy(xts[n][:, c, ts(i4, 128)], pt)

        def stage3(n):
            # out.T columns of chunk n -> [b, d] layout -> DRAM
            for i in range(4 * n, 4 * n + 4):
                pt = psT.tile([128, 512], f32, name="pt")[:, :64]
                nc.tensor.transpose(pt, outT[:, ts(i, 128)], ident[:64, :64])
                ob = outp.tile([128, D], f32, name="ob")
                nc.vector.tensor_copy(ob[:], pt)
                nc.gpsimd.dma_start(out_d[ts(i, 128), :], ob[:])

        def body():
            stage1(0, pool=(psA, "pa"))
            # ---- stage 2 per chunk: MM1 -> exp/ln -> S0 -> exp -> MM3 ----
            # Consecutive tree-pairs (g, g+1) share one 1024-wide U tile so
            # the final exp runs half as many, larger ACT instructions.
            for n in range(NBC):
                if n + 1 < NBC:
                    stage1(n + 1)
                nlq_tiles = []
                for m2 in range(NMT // 2):
                    et = expp.tile([MROWS, 2 * BC], f32, name="et")
                    for mh in range(2):
                        m = 2 * m2 + mh
                        pl = psA.tile([128, 512], f32, name="pa")[:MROWS, :BC]
                        for c in range(NFC):
                            nc.tensor.matmul(
                                pl,
                                wpm[m][:, c, :],
                                xts[n][:, c, :],
                                start=(c == 0),
                                stop=(c == NFC - 1),
                            )
                        nc.scalar.activation(
                            et[:, ts(mh, BC)], pl, AF.Exp,
                            bias=bp[:, m : m + 1], scale=1.0,
                        )
                    nt = nlqp.tile([MROWS, 2 * BC], MM_DT, name="nlq")
                    nc.scalar.activation(nt[:], et[:], AF.Ln, bias=1.0, scale=1.0)
                    nlq_tiles.append(nt)
                pon = psO.tile([64, BC], f32, name="pon")
                for g2 in range(NPAIR // 2):
                    pu = psU.tile([128, 2 * BC], f32, name="pu")
                    pth = pathp.tile([128, 2 * BC], MM_DT, name="path")
                    for h in range(2):
                        g = 2 * g2 + h
                        m, p = g // 4, g % 4
                        nc.tensor.matmul(
                            pu[:, ts(h, BC)],
                            s0[:, p, :],
                            nlq_tiles[m // 2][:, ts(m % 2, BC)],
                            start=True,
                            stop=True,
                        )
                    nc.scalar.activation(pth[:], pu[:], AF.Exp, scale=-1.0)
                    for h in range(2):
                        g = 2 * g2 + h
                        nc.tensor.matmul(
                            pon[:],
                            leaf[:, g, :],
                            pth[:, ts(h, BC)],
                            start=(g == 0),
                            stop=(g == NPAIR - 1),
                        )
                nc.vector.tensor_copy(outT[:, ts(n, BC)], pon[:])
                stage3(n)

        if hw_loop:
            with tc.For_i(0, hw_loop, 1):
                body()
        else:
            for _rep in range(repeat):
                body()


_ACT_TABLES_PATCHED = False


def _patch_act_tables():
    """Force every activation to resolve to natural_log_exp_and_others.

    The default first-fit table choice sends Exp to `exp_and_others` and Ln
    to `natural_log`, inserting a 1.3us LoadActFuncSet at every Exp<->Ln
    transition (57 loads = 73us on the ACT engine).  One table covers all
    functions this kernel uses (Exp, Ln, Copy, Identity), so blank out the
    alternatives (preserving list order, which is the act_func_set_id space).
    """
    global _ACT_TABLES_PATCHED
    if _ACT_TABLES_PATCHED:
        return
    import concourse.bacc as bacc_mod

    orig = bacc_mod.get_activation_tables
    keep = "natural_log_exp_and_others"

    def filtered(arch):
        t = orig(arch)
        assert keep in t
        return {k: (v if k == keep else set()) for k, v in t.items()}

    bacc_mod.get_activation_tables = filtered
    _ACT_TABLES_PATCHED = True


def build_nc(repeat=1, hw_loop=0):
    _patch_act_tables()
    nc = bacc.Bacc("TRN2", target_bir_lowering=False, debug=False)
    x_d = nc.dram_tensor("x", [B, F], f32, kind="ExternalInput").ap()
    wp_d = nc.dram_tensor("wp", [F, WCOLS], MM_DT, kind="ExternalInput").ap()
    bp_d = nc.dram_tensor("bp", [MROWS, NMT], f32, kind="ExternalInput").ap()
    s0_d = nc.dram_tensor("s0", [4 * 128, 128], MM_DT, kind="ExternalInput").ap()
    leaf_d = nc.dram_tensor("leaf", [T * L, D], MM_DT, kind="ExternalInput").ap()
    ident_d = nc.dram_tensor("ident", [128, 128], f32, kind="ExternalInput").ap()
    out_d = nc.dram_tensor("out", [B, D], f32, kind="ExternalOutput").ap()
    _build_kernel(
        nc,
        (x_d, wp_d, bp_d, s0_d, leaf_d, ident_d, out_d),
        repeat=repeat,
        hw_loop=hw_loop,
    )
    nc.compile()
    return nc


def host_prep(W, b, leaf_values):
    """Rearrange parameters for the device kernel (pure data movement)."""
    W = np.asarray(W, np.float32)
    b = np.asarray(b, np.float32)
    leaf_values = np.asarray(leaf_values, np.float32)
    Wp = np.zeros((F, WCOLS), np.float32)
    bp = np.zeros((WCOLS,), np.float32)
    for g in range(NPAIR):
        for tloc in range(2):
            t = 2 * g + tloc
            for k in range(DEPTH):
                for j in range(2):
                    col = g * PBLK + tloc * 12 + k * 2 + j
                    s = float(2 * j - 1)
                    Wp[:, col] = s * W[k, :, t]
                    bp[col] = s * b[k, t]
    bp = bp.reshape(NMT, MROWS).T.copy()  # [128, 8], column m = rows of tile m
    # 4 variants of the selection matrix; variant p contracts the pair block
    # at partition offset 32*p of an NLQ tile.
    S0p = np.zeros((4, 128, 128), np.float32)
    for p in range(4):
        for tloc in range(2):
            for k in range(DEPTH):
                for j in range(2):
                    row = 32 * p + tloc * 12 + k * 2 + j
                    for l in range(L):
                        if ((l >> (DEPTH - 1 - k)) & 1) == j:
                            S0p[p, row, tloc * 64 + l] = 1.0
    S0p = S0p.reshape(4 * 128, 128)
    leaf_flat = (leaf_values / float(T)).reshape(T * L, D).astype(np.float32)
    ident = np.eye(128, dtype=np.float32)
    return Wp, bp, S0p, leaf_flat, ident


_NC = None


def kernel(x, W, b, leaf_values):
    global _NC
    x = np.asarray(x, np.float32)
    Wp, bp, S0p, leaf_flat, ident = host_prep(W, b, leaf_values)
    if _NC is None:
        _NC = build_nc()
    in_maps = [
        {
            "x": np.ascontiguousarray(x[i * B : (i + 1) * B]),
            "wp": Wp,
            "bp": bp,
            "s0": S0p,
            "leaf": leaf_flat,
            "ident": ident,
        }
        for i in range(N_CORES)
    ]
    from concourse.bass_utils import run_bass_kernel_spmd

    res = run_bass_kernel_spmd(_NC, in_maps, list(range(N_CORES)))
    return np.concatenate([res.results[i]["out"] for i in range(N_CORES)], axis=0)
